# revision 2
# baseline (speedup 1.0000x reference)
"""GAT layer kernel for 8 TRN2 NeuronCores (self-contained).

Sharding: core c handles batch b = c//2 and head-pair (2*(c%2), 2*(c%2)+1).
Each core computes heads_h = softmax(leaky(s_i+s_j) + bias) @ t_h for its two
heads and returns their scaled sum; the host adds the two cores per batch.

Device pipeline per (i-tile, head), i on partitions:
  PE   : PSUM_A = rank-4(w = s_i + s_j, split-fp16) + I @ bias  (additive mask)
  ACT  : L = Prelu(PSUM_A, alpha=0.2)            -> SBUF f32  (masked leaky scores)
  DVE  : m = rowmax(L)                                        (exact masked max)
  ACT  : E = Exp(L - m) -> f16, accum_out = Z                 (masked row sums)
  DMA  : transpose E into 16 [128,128] blocks (j on partitions)
  PE   : OUT += E_J^T.T @ t_J  over j-blocks
  DVE  : out = OUT * 1/(4Z)  (+ other head's contribution)
"""
import numpy as np
import ml_dtypes

B, N, F_IN, F_OUT, H = 4, 2048, 256, 256, 4
P = 128
NT = N // P  # 16

_NC = None


def _build():
    import concourse.tile as tile
    from concourse import bacc, mybir

    dt = mybir.dt
    f32, f16, bf16 = dt.float32, dt.float16, dt.bfloat16
    AF = mybir.ActivationFunctionType
    ALU = mybir.AluOpType

    nc = bacc.Bacc("TRN2", target_bir_lowering=False, debug=False, num_devices=8)

    d_xhi = nc.dram_tensor("xhi", [F_IN, N], f16, kind="ExternalInput").ap()
    d_xlo = nc.dram_tensor("xlo", [F_IN, N], f16, kind="ExternalInput").ap()
    d_w = nc.dram_tensor("w", [2 * F_IN, F_OUT], f16, kind="ExternalInput").ap()
    d_wa = nc.dram_tensor("wa", [P, 8], f16, kind="ExternalInput").ap()
    d_bvec = nc.dram_tensor("bvec", [1, 2 * F_OUT], f16, kind="ExternalInput").ap()
    d_ba = nc.dram_tensor("ba", [1, 2], f32, kind="ExternalInput").ap()
    d_ident = nc.dram_tensor("ident", [P, P], bf16, kind="ExternalInput").ap()
    d_bias = nc.dram_tensor("biasm", [N, N], bf16, kind="ExternalInput").ap()
    d_out = nc.dram_tensor("out", [N, F_OUT], f32, kind="ExternalOutput").ap()

    with tile.TileContext(nc) as tc:
        with tc.tile_pool(name="constp", bufs=1) as constp, \
             tc.tile_pool(name="xpool", bufs=1) as xpool, \
             tc.tile_pool(name="tpool", bufs=1) as tpool, \
             tc.tile_pool(name="rowp", bufs=1) as rowp, \
             tc.tile_pool(name="work", bufs=2) as work:

            ident = constp.tile([P, P], bf16)
            nc.sync.dma_start(ident[:], d_ident[:])
            alpha_t = constp.tile([P, 1], f32)
            nc.gpsimd.memset(alpha_t[:], 0.2)
            ones_row = constp.tile([1, N], f16)
            nc.gpsimd.memset(ones_row[:], 1.0)
            wa_sb = constp.tile([P, 8], f16)
            nc.sync.dma_start(wa_sb[:], d_wa[:])
            bvec_sb = constp.tile([1, 2 * F_OUT], f16)
            nc.sync.dma_start(bvec_sb[:], d_bvec[:])
            ba_sb = constp.tile([1, 2], f32)
            nc.sync.dma_start(ba_sb[:], d_ba[:])

            xhi = [xpool.tile([P, N], f16, name=f"xhi{c}") for c in range(2)]
            xlo = [xpool.tile([P, N], f16, name=f"xlo{c}") for c in range(2)]
            for c in range(2):
                nc.sync.dma_start(xhi[c][:], d_xhi[c * P:(c + 1) * P, :])
                nc.sync.dma_start(xlo[c][:], d_xlo[c * P:(c + 1) * P, :])
            wsb = [[xpool.tile([P, F_OUT], f16, name=f"wsb{h}_{c}") for c in range(2)]
                   for h in range(2)]
            for h in range(2):
                for c in range(2):
                    nc.sync.dma_start(wsb[h][c][:],
                                      d_w[h * F_IN + c * P: h * F_IN + (c + 1) * P, :])

            t_tiles = [[tpool.tile([P, F_OUT], f16, name=f"t{h}_{J}") for J in range(NT)]
                       for h in range(2)]
            LT4 = [rowp.tile([4, N], f16, name=f"LT4_{h}") for h in range(2)]
            RT4 = [rowp.tile([4, N], f16, name=f"RT4_{h}") for h in range(2)]

            with tc.tile_pool(name="pss", bufs=2, space="PSUM") as pss:
                for h in range(2):
                    # ---- s = x @ (W a) + b.a  (split-fp16, M=1 matmuls) ----
                    s_row = rowp.tile([1, N], f32, name=f"s_row{h}")
                    for q in range(4):
                        sl = slice(q * 512, (q + 1) * 512)
                        s_ps = pss.tile([1, 512], f32, name=f"s_ps{h}_{q}", tag="s_ps")
                        pieces = [(0, 0), (0, 1), (1, 0)]  # (x split, wa split)
                        n_mm = len(pieces) * 2
                        k = 0
                        for xs, ws in pieces:
                            xt = xhi if xs == 0 else xlo
                            for kc in range(2):
                                col = h * 4 + ws * 2 + kc
                                nc.tensor.matmul(
                                    s_ps[0:1, :], wa_sb[:, col:col + 1], xt[kc][:, sl],
                                    start=(k == 0), stop=(k == n_mm - 1))
                                k += 1
                        nc.scalar.activation(s_row[0:1, sl], s_ps[0:1, :], AF.Identity,
                                             bias=ba_sb[0:1, h:h + 1], scale=1.0)
                    s_hi_row = rowp.tile([1, N], f16, name=f"s_hi_row{h}")
                    nc.scalar.activation(s_hi_row[:], s_row[:], AF.Identity)
                    s_rem = rowp.tile([1, N], f32, name=f"s_rem{h}")
                    nc.vector.tensor_sub(s_rem[:], s_row[:], s_hi_row[:])
                    s_lo_row = rowp.tile([1, N], f16, name=f"s_lo_row{h}")
                    nc.scalar.activation(s_lo_row[:], s_rem[:], AF.Identity)

                    nc.sync.dma_start(LT4[h][0:1, :], s_hi_row[:])
                    nc.sync.dma_start(LT4[h][1:2, :], s_lo_row[:])
                    nc.sync.dma_start(LT4[h][2:3, :], ones_row[:])
                    nc.sync.dma_start(LT4[h][3:4, :], ones_row[:])
                    nc.sync.dma_start(RT4[h][0:1, :], ones_row[:])
                    nc.sync.dma_start(RT4[h][1:2, :], ones_row[:])
                    nc.sync.dma_start(RT4[h][2:3, :], s_hi_row[:])
                    nc.sync.dma_start(RT4[h][3:4, :], s_lo_row[:])

                    # ---- t_h = x @ W_h + b_h  (node-major f16 tiles) ----
                    for J in range(NT):
                        t_ps = pss.tile([P, F_OUT], f32, name=f"t_ps{h}_{J}", tag="t_ps")
                        jsl = slice(J * P, (J + 1) * P)
                        nc.tensor.matmul(t_ps[:], xhi[0][:, jsl], wsb[h][0][:],
                                         start=True, stop=False)
                        nc.tensor.matmul(t_ps[:], xhi[1][:, jsl], wsb[h][1][:],
                                         start=False, stop=False)
                        nc.tensor.matmul(t_ps[:], ones_row[0:1, 0:P],
                                         bvec_sb[0:1, h * F_OUT:(h + 1) * F_OUT],
                                         start=False, stop=True)
                        nc.vector.tensor_copy(t_tiles[h][J][:], t_ps[:])

            # ---- main loop over i-tiles ----
            with tc.tile_pool(name="psA", bufs=1, space="PSUM") as psA, \
                 tc.tile_pool(name="psO", bufs=2, space="PSUM") as psO:
              for I in range(NT):
                  isl = slice(I * P, (I + 1) * P)
                  btile = work.tile([P, N], bf16, name=f"btile{I}", tag="btile")
                  nc.sync.dma_start(btile[:], d_bias[isl, :])
                  acc = work.tile([P, F_OUT], f32, name=f"acc{I}", tag="acc")
                  for h in range(2):
                      A = psA.tile([P, N], f32, name=f"A{I}_{h}", tag="A")
                      for q in range(4):
                          sl = slice(q * 512, (q + 1) * 512)
                          nc.tensor.matmul(A[:, sl], LT4[h][:, isl], RT4[h][:, sl],
                                           start=True, stop=False)
                          nc.tensor.matmul(A[:, sl], ident[:], btile[:, sl],
                                           start=False, stop=True)
                      L = work.tile([P, N], f32, name=f"L{I}_{h}", tag="L")
                      nc.scalar.activation(L[:], A[:], AF.Prelu,
                                           bias=0.0, scale=1.0, alpha=alpha_t[:])
                      m_col = work.tile([P, 1], f32, name=f"m{I}_{h}", tag="mcol")
                      nc.vector.tensor_reduce(m_col[:], L[:], axis=mybir.AxisListType.X,
                                              op=ALU.max)
                      nm = work.tile([P, 1], f32, name=f"nm{I}_{h}", tag="nm")
                      nc.vector.tensor_scalar_mul(nm[:], m_col[:], -1.0)
                      E = work.tile([P, N], f16, name=f"E{I}_{h}", tag="E")
                      Z = work.tile([P, 1], f32, name=f"Z{I}_{h}", tag="Z")
                      nc.scalar.activation(E[:], L[:], AF.Exp,
                                           bias=nm[:], scale=1.0, accum_out=Z[:])
                      ET = work.tile([P, N], f16, name=f"ET{I}_{h}", tag="ET")
                      et3 = ET[:].rearrange("p (J f) -> p J f", f=P)
                      eng = nc.sync if h == 0 else nc.scalar
                      eng.dma_start_transpose(et3, E[:])
                      O = psO.tile([P, F_OUT], f32, name=f"O{I}_{h}", tag="O")
                      for J in range(NT):
                          jsl = slice(J * P, (J + 1) * P)
                          nc.tensor.matmul(O[:], ET[:, jsl], t_tiles[h][J][:],
                                           start=(J == 0), stop=(J == NT - 1))
                      z4 = work.tile([P, 1], f32, name=f"z4{I}_{h}", tag="z4")
                      nc.vector.tensor_scalar_mul(z4[:], Z[:], 4.0)
                      rz = work.tile([P, 1], f32, name=f"rz{I}_{h}", tag="rz")
                      nc.vector.reciprocal(rz[:], z4[:])
                      if h == 0:
                          nc.vector.tensor_scalar(acc[:], O[:], rz[:], None,
                                                  op0=ALU.mult)
                      else:
                          nc.vector.scalar_tensor_tensor(acc[:], O[:], rz[:], acc[:],
                                                         op0=ALU.mult, op1=ALU.add)
                  nc.sync.dma_start(d_out[isl, :], acc[:])

    nc.compile()
    return nc


def prepare_in_maps(inputs, bias, W, a, b):
    inputs = np.asarray(inputs, dtype=np.float32)
    bias = np.asarray(bias, dtype=np.float32)
    W = np.asarray(W, dtype=np.float32)
    a = np.asarray(a, dtype=np.float32)
    b = np.asarray(b, dtype=np.float32)

    ident = np.eye(P, dtype=ml_dtypes.bfloat16)
    in_maps = []
    for c in range(8):
        bb = c // 2
        hp = c % 2
        hs = [2 * hp, 2 * hp + 1]
        xT = np.ascontiguousarray(inputs[bb].T)            # [F_IN, N] f32
        xhi = xT.astype(np.float16)
        xlo = (xT - xhi.astype(np.float32)).astype(np.float16)
        Wp = np.concatenate([W[hs[0]], W[hs[1]]], axis=0).astype(np.float16)
        Wa = np.einsum('hfo,ho->hf', W[hs].astype(np.float64),
                       a[hs].astype(np.float64))           # [2, F_IN]
        Wahi = Wa.astype(np.float16)
        Walo = (Wa - Wahi.astype(np.float64)).astype(np.float16)
        wa_pack = np.zeros((P, 8), np.float16)
        for h in range(2):
            for s_, arr in enumerate([Wahi, Walo]):
                for kc in range(2):
                    wa_pack[:, h * 4 + s_ * 2 + kc] = arr[h, kc * P:(kc + 1) * P]
        bvec = np.concatenate([b[hs[0]], b[hs[1]]]).astype(np.float16)[None, :]
        ba = np.array([[float(np.dot(b[hs[0]].astype(np.float64), a[hs[0]])),
                        float(np.dot(b[hs[1]].astype(np.float64), a[hs[1]]))]],
                      np.float32)
        biasm = bias[bb].astype(ml_dtypes.bfloat16)
        in_maps.append(dict(xhi=xhi, xlo=xlo, w=Wp, wa=wa_pack, bvec=bvec,
                            ba=ba, ident=ident, biasm=biasm))
    return in_maps


def gather_output(results):
    outs = [results[c]["out"] for c in range(8)]
    out = np.stack([outs[2 * bb] + outs[2 * bb + 1] for bb in range(B)])
    return out.astype(np.float32)


def get_nc():
    global _NC
    if _NC is None:
        _NC = _build()
    return _NC


_LAST_EXEC_NS = None
_LAST_TRACE = None


def kernel(inputs, bias, W, a, b):
    global _LAST_EXEC_NS, _LAST_TRACE
    from concourse.bass_utils import run_bass_kernel_spmd
    nc = get_nc()
    in_maps = prepare_in_maps(inputs, bias, W, a, b)
    res = run_bass_kernel_spmd(nc, in_maps, core_ids=list(range(8)))
    _LAST_EXEC_NS = res.exec_time_ns
    _LAST_TRACE = res.instructions_and_trace
    return gather_output(res.results)

